# revision 66
# baseline (speedup 1.0000x reference)
"""Trainium2 Bass kernel for additive-attention scoring (Bahdanau-style).

Math (H=512, B=64, S=2048):
    ref_t[b,o,s] = sum_h Wr[o,h] * ref[b,s,h] + br[o]          # [B,H,S]
    q[b,o]      = sum_h Wq[o,h] * query[b,h] + bq[o]           # [B,H]
    logits[b,s] = sum_o V[o] * tanh(q[b,o] + ref_t[b,o,s])     # [B,S]

Strategy: data-parallel over batch across 8 NeuronCores (8 batches per
core), params replicated.  The host lays the per-core ref shard out
chunk-major ([32 chunks, 128, 4, 513] — contraction dim on partitions,
tanh bias bundled as a 513th column) so every device DMA is one fully
contiguous ~1MB transfer.  The device program is a small For_i hardware
loop (the per-instruction fetch cost on this stack is ~3.5us for
straight-line code, so the loop body must stay resident in the ISA
cache).  Matmuls run as float32r (full-rate fp32), the tanh runs on
ScalarE with a per-partition bias, and the V-reduction over output
channels is a TensorE matvec in bf16.
"""

import sys

import numpy as np

if "/opt/trn_rl_repo" not in sys.path:  # grading env safety
    sys.path.insert(0, "/opt/trn_rl_repo")

import concourse.bass as bass
import concourse.tile as tile
from concourse import bacc, mybir
from concourse.bass import ds
from concourse.bass_utils import run_bass_kernel_spmd

B, S, H = 64, 2048, 512
NCORES = 8
BC = B // NCORES        # batches per core
P = 128                 # partitions
HT = H // P             # 4 k-tiles (contraction)
OT = H // P             # 4 output-channel tiles
SCH = 512               # s-chunk (one PSUM bank of fp32)
NS = S // SCH           # 4 s-chunks per batch
NCH = BC * NS           # 32 chunks per core
U = 8                   # chunks unrolled per For_i iteration

F32 = mybir.dt.float32
F32R = mybir.dt.float32r
BF16 = mybir.dt.bfloat16


def _build_program(reps=1, mode="full", unroll=U, out_eng="scalar",
                   staggered=False, dtype="f32", pipe=False):
    nc = bacc.Bacc("TRN2", target_bir_lowering=False, debug=False)

    bf = dtype == "bf16"
    mm_dt = BF16 if bf else F32R
    payload = 514 if bf else 513
    refC = nc.dram_tensor(
        "refC", [NCH, P, HT, payload], BF16 if bf else F32, kind="ExternalInput"
    ).ap()
    WrT = nc.dram_tensor(
        "WrT", [H, H], BF16 if bf else F32, kind="ExternalInput"
    ).ap()
    brt = nc.dram_tensor("brt", [P, OT], F32, kind="ExternalInput").ap()
    vt = nc.dram_tensor("vt", [P, OT], BF16, kind="ExternalInput").ap()

    refoutC = nc.dram_tensor(
        "refoutC", [NCH, P, OT, SCH], F32, kind="ExternalOutput"
    ).ap()
    logitsC = nc.dram_tensor("logitsC", [NCH, SCH], F32, kind="ExternalOutput").ap()

    Tanh = mybir.ActivationFunctionType.Tanh

    with tile.TileContext(nc) as tc:
        nbuf = min(unroll + 1, 8)
        with (
            tc.tile_pool(name="const", bufs=1) as const,
            tc.tile_pool(name="io", bufs=nbuf) as io,
            tc.tile_pool(name="outp", bufs=nbuf) as outp,
            tc.tile_pool(name="thp", bufs=3) as thp,
            tc.tile_pool(name="lgp", bufs=3) as lgp,
            tc.tile_pool(name="psum", bufs=6, space="PSUM") as psum,
            tc.tile_pool(name="psl", bufs=2, space="PSUM") as psl,
        ):
            wr_sb = const.tile([P, HT, H], mm_dt, name="wr_sb")
            nc.sync.dma_start(
                wr_sb[:], WrT.rearrange("(ht hp) o -> hp ht o", hp=P).bitcast(mm_dt)
            )
            brt_sb = const.tile([P, OT], F32, name="brt_sb")
            nc.sync.dma_start(brt_sb[:], brt)
            vt_sb = const.tile([P, OT], BF16, name="vt_sb")
            nc.sync.dma_start(vt_sb[:], vt)

            out_dma = getattr(nc, out_eng).dma_start

            if pipe:
                Tanh_ = Tanh

                def _load(pp, iv):
                    c = iv & (NCH - 1)
                    rs = pp.intermediate_tile([P, HT, payload], mm_dt)
                    nc.sync.dma_start(rs[:], refC[ds(c, 1)].bitcast(mm_dt))
                    return rs

                def _compute(pp, iv, rs):
                    out_sb = pp.intermediate_tile([P, OT, SCH], F32)
                    lg = pp.intermediate_tile([1, SCH], F32)
                    ps_l = psl.tile([1, SCH], F32, name="ps_l", tag="psl")
                    for ot in range(OT):
                        ps = psum.tile([P, SCH], F32, name="ps", tag="ps")
                        for ht in range(HT):
                            nc.tensor.matmul(
                                ps[:],
                                wr_sb[:, ht, ot * P : (ot + 1) * P],
                                rs[:, ht, 0:SCH],
                                start=(ht == 0),
                                stop=(ht == HT - 1),
                            )
                        nc.vector.tensor_scalar_add(
                            out_sb[:, ot, :], ps[:], brt_sb[:, ot : ot + 1]
                        )
                        th = thp.tile([P, SCH], BF16, name="th", tag="th")
                        nc.scalar.activation(
                            th[:], ps[:], Tanh_,
                            bias=rs[:, ot, 512:513].bitcast(
                                BF16 if bf else F32),
                        )
                        nc.tensor.matmul(
                            ps_l[:],
                            vt_sb[:, ot : ot + 1],
                            th[:],
                            start=(ot == 0),
                            stop=(ot == OT - 1),
                        )
                    nc.vector.tensor_copy(lg[:], ps_l[:])
                    return (out_sb, lg)

                def _store(pp, iv, tiles):
                    out_sb, lg = tiles
                    c = iv & (NCH - 1)
                    nc.sync.dma_start(logitsC[ds(c, 1)], lg[:])
                    out_dma(refoutC[ds(c, 1)], out_sb[:])

                tc.For_i_pipelined(
                    [_load, _compute, _store], 0, NCH * reps, 1,
                    unroll=unroll, staggered_reset=staggered,
                )
                n_iter = 0
            else:
                n_iter = (NCH // unroll) * reps
            if not pipe:
              with tc.For_i(0, n_iter, 1, staggered_reset=staggered) as iv:
                for u in range(unroll):
                    c = (iv * unroll + u) & (NCH - 1)

                    rs = io.tile([P, HT, payload], mm_dt, name="rs", tag="rs")
                    nc.sync.dma_start(rs[:], refC[ds(c, 1)].bitcast(mm_dt))

                    if mode == "dma":
                        assert not bf
                        out_dma(
                            refoutC[ds(c, 1)], rs[:, :, 0:SCH].bitcast(F32)
                        )
                        continue

                    out_sb = outp.tile([P, OT, SCH], F32, name="out_sb",
                                       tag="out_sb")
                    ps_l = psl.tile([1, SCH], F32, name="ps_l", tag="psl")

                    for ot in range(OT):
                        ps = psum.tile([P, SCH], F32, name="ps", tag="ps")
                        for ht in range(HT):
                            nc.tensor.matmul(
                                ps[:],
                                wr_sb[:, ht, ot * P : (ot + 1) * P],
                                rs[:, ht, 0:SCH],
                                start=(ht == 0),
                                stop=(ht == HT - 1),
                            )
                        # ref_t tile = mm + br  (per-partition bias on DVE)
                        nc.vector.tensor_scalar_add(
                            out_sb[:, ot, :], ps[:], brt_sb[:, ot : ot + 1]
                        )
                        # tanh(mm + q + bq + br) on ScalarE, bias bundled in rs
                        th = thp.tile([P, SCH], BF16, name="th", tag="th")
                        nc.scalar.activation(
                            th[:],
                            ps[:],
                            Tanh,
                            bias=rs[:, ot, 512:513].bitcast(BF16 if bf else F32),
                        )
                        # logits partial: V[o-tile] . tanh  (TensorE matvec)
                        nc.tensor.matmul(
                            ps_l[:],
                            vt_sb[:, ot : ot + 1],
                            th[:],
                            start=(ot == 0),
                            stop=(ot == OT - 1),
                        )

                    lg = lgp.tile([1, SCH], F32, name="lg", tag="lg")
                    nc.vector.tensor_copy(lg[:], ps_l[:])
                    nc.sync.dma_start(logitsC[ds(c, 1)], lg[:])
                    out_dma(refoutC[ds(c, 1)], out_sb[:])

    nc.compile()
    return nc


_NC_CACHE = {}


def _get_program(reps=1, mode="full", unroll=U, out_eng="scalar",
                 staggered=False, dtype="f32", pipe=False):
    key = (reps, mode, unroll, out_eng, staggered, dtype, pipe)
    if key not in _NC_CACHE:
        _NC_CACHE[key] = _build_program(reps, mode, unroll, out_eng, staggered,
                                        dtype, pipe)
    return _NC_CACHE[key]


def _prep_in_maps(query, ref, Wq, bq, Wr, br, V, dtype="f32"):
    import ml_dtypes

    bf = dtype == "bf16"
    query = np.ascontiguousarray(query, dtype=np.float32)
    ref = np.ascontiguousarray(ref, dtype=np.float32)
    Wq = np.asarray(Wq, dtype=np.float32)
    Wr = np.asarray(Wr, dtype=np.float32)
    bq = np.asarray(bq, dtype=np.float32)
    br = np.asarray(br, dtype=np.float32)
    V = np.asarray(V, dtype=np.float32)

    WrT = np.ascontiguousarray(Wr.T)
    if bf:
        WrT = WrT.astype(ml_dtypes.bfloat16)
    brt = np.ascontiguousarray(br.reshape(OT, P).T)
    vt = np.ascontiguousarray(V.reshape(OT, P).T).astype(ml_dtypes.bfloat16)

    # host-side q head (0.05% of the FLOPs)
    qbias = query @ Wq.T + bq + br                      # [B, H]

    # chunk-major ref layout: [b,s,h] -> [c=(b,si), hp, ht, s] + bias col(s)
    refT = ref.reshape(B, S, HT, P)                     # [b, s, ht, hp]
    # -> [b, si, hp, ht, s_in]
    refCm = refT.reshape(B, NS, SCH, HT, P).transpose(0, 1, 4, 3, 2)
    refCm = refCm.reshape(B * NS, P, HT, SCH)
    qb = qbias.reshape(B, OT, P).transpose(0, 2, 1)     # [b, hp, ot]
    qb = np.repeat(qb[:, None, :, :], NS, axis=1).reshape(B * NS, P, OT, 1)
    pad = [refCm, qb]
    if bf:
        pad.append(np.zeros((B * NS, P, HT, 1), np.float32))
    refC_all = np.concatenate(pad, axis=3)              # [B*NS, P, HT, 513/514]
    refC_all = np.ascontiguousarray(refC_all)
    if bf:
        refC_all = refC_all.astype(ml_dtypes.bfloat16)

    in_maps = []
    for c in range(NCORES):
        cs = slice(c * NCH, (c + 1) * NCH)
        in_maps.append(
            {"refC": refC_all[cs], "WrT": WrT, "brt": brt, "vt": vt}
        )
    return in_maps


def _gather(results):
    ref_t_parts = []
    logits_parts = []
    for c in range(NCORES):
        ro = results[c]["refoutC"]                      # [NCH, P, OT, SCH]
        # [b, si, op, ot, s] -> [b, ot, op, si, s]
        ro = ro.reshape(BC, NS, P, OT, SCH).transpose(0, 3, 2, 1, 4)
        ref_t_parts.append(ro.reshape(BC, H, S))
        logits_parts.append(results[c]["logitsC"].reshape(BC, S))
    return np.concatenate(ref_t_parts, 0), np.concatenate(logits_parts, 0)


DTYPE = "f32"


def run_with_results(trace=False, **inputs):
    """Run on 8 NeuronCores; returns ((ref_t, logits), BassKernelResults)."""
    nc = _get_program(dtype=DTYPE)
    in_maps = _prep_in_maps(**inputs, dtype=DTYPE)
    res = run_bass_kernel_spmd(nc, in_maps, core_ids=list(range(NCORES)), trace=trace)
    ref_t, logits = _gather(res.results)
    return (ref_t, logits), res


def kernel(**inputs):
    outs, _ = run_with_results(trace=False, **inputs)
    return outs


def bench(n_iters=32, reps=1, mode="full", unroll=U, out_eng="scalar",
          staggered=False, dtype="f32", pipe=False, **inputs):
    """Measure per-iteration device time by queueing n_iters back-to-back
    executions (outputs of iter i are donated as output buffers of iter
    i+1, so there is no per-iter memset/transfer)."""
    import time

    import jax
    from jax.experimental.shard_map import shard_map
    from jax.sharding import Mesh, NamedSharding, PartitionSpec

    from concourse import bass2jax
    from concourse.bass2jax import _bass_exec_p, partition_id_tensor

    nc = _get_program(reps, mode, unroll, out_eng, staggered, dtype, pipe)
    in_maps = _prep_in_maps(**inputs, dtype=dtype)
    bass2jax.install_neuronx_cc_hook()

    import concourse.mybir as _mybir

    partition_name = nc.partition_id_tensor.name if nc.partition_id_tensor else None
    in_names, out_names, out_avals, zero_outs = [], [], [], []
    for alloc in nc.m.functions[0].allocations:
        if not isinstance(alloc, _mybir.MemoryLocationSet):
            continue
        name = alloc.memorylocations[0].name
        if alloc.kind == "ExternalInput":
            if name != partition_name:
                in_names.append(name)
        elif alloc.kind == "ExternalOutput":
            shape = tuple(alloc.tensor_shape)
            dtype = _mybir.dt.np(alloc.dtype)
            out_names.append(name)
            out_avals.append(jax.core.ShapedArray(shape, dtype))
            zero_outs.append(np.zeros(shape, dtype))
    n_params = len(in_names)
    n_outs = len(out_avals)
    in_names = in_names + out_names
    if partition_name is not None:
        in_names.append(partition_name)

    def _body(*args):
        operands = list(args)
        if partition_name is not None:
            operands.append(partition_id_tensor())
        return tuple(
            _bass_exec_p.bind(
                *operands,
                out_avals=tuple(out_avals),
                in_names=tuple(in_names),
                out_names=tuple(out_names),
                lowering_input_output_aliases=(),
                sim_require_finite=True,
                sim_require_nnan=True,
                nc=nc,
            )
        )

    devices = jax.devices()[:NCORES]
    mesh = Mesh(np.asarray(devices), ("core",))
    in_specs = (PartitionSpec("core"),) * (n_params + n_outs)
    out_specs = (PartitionSpec("core"),) * n_outs
    donate = tuple(range(n_params, n_params + n_outs))
    sharded = jax.jit(
        shard_map(_body, mesh=mesh, in_specs=in_specs, out_specs=out_specs,
                  check_rep=False),
        donate_argnums=donate,
        keep_unused=True,
    )

    per_core = [
        [np.asarray(m[name]) for name in in_names[:n_params]] for m in in_maps
    ]
    concat_in = [
        np.concatenate([per_core[c][i] for c in range(NCORES)], axis=0)
        for i in range(n_params)
    ]
    concat_zeros = [
        np.zeros((NCORES * z.shape[0], *z.shape[1:]), z.dtype) for z in zero_outs
    ]
    shard = NamedSharding(mesh, PartitionSpec("core"))
    concat_in = [jax.device_put(a, shard) for a in concat_in]
    outs = sharded(*concat_in, *concat_zeros)  # warmup + compile
    jax.block_until_ready(outs)

    t0 = time.time()
    for _ in range(n_iters):
        outs = sharded(*concat_in, *outs)
    jax.block_until_ready(outs)
    t1 = time.time()
    return (t1 - t0) / n_iters


# revision 75
# speedup vs baseline: 1.9440x; 1.9440x over previous
"""Trainium2 Bass kernel for additive-attention scoring (Bahdanau-style).

Math (H=512, B=64, S=2048):
    ref_t[b,o,s] = sum_h Wr[o,h] * ref[b,s,h] + br[o]          # [B,H,S]
    q[b,o]      = sum_h Wq[o,h] * query[b,h] + bq[o]           # [B,H]
    logits[b,s] = sum_o V[o] * tanh(q[b,o] + ref_t[b,o,s])     # [B,S]

Strategy: data-parallel over batch across 8 NeuronCores (8 batches per
core), params replicated.  The host lays the per-core ref shard out
chunk-major ([32 chunks, 128 partitions, 4 k-tiles, payload] — the
contraction dim on partitions, the tanh bias bundled as an extra
column) so every device DMA is one fully contiguous ~1MB transfer.  The
device program is a single-iteration For_i hardware-loop body (the
per-instruction fetch cost on this stack is ~3.5us for straight-line
code, so all work must execute from the ISA cache; the loop exists so
benchmark builds can repeat the workload in-NEFF).  The matmul dtype is
bf16 by default (DTYPE below; inputs are rounded host-side, fp32 PSUM
accumulation, rel err ~2e-3 vs the 2e-2 gate) which halves the input
DMA bytes; set DTYPE="f32" for a float32r (full-rate fp32, rel err
1.3e-4) variant at ~40% more runtime.  The tanh runs on ScalarE with a
per-partition bias, and the V-reduction over output channels is a
TensorE matvec in bf16.
"""

import sys

import numpy as np

if "/opt/trn_rl_repo" not in sys.path:  # grading env safety
    sys.path.insert(0, "/opt/trn_rl_repo")

import concourse.bass as bass
import concourse.tile as tile
from concourse import bacc, mybir
from concourse.bass import ds
from concourse.bass_utils import run_bass_kernel_spmd

B, S, H = 64, 2048, 512
NCORES = 8
BC = B // NCORES        # batches per core
P = 128                 # partitions
HT = H // P             # 4 k-tiles (contraction)
OT = H // P             # 4 output-channel tiles
SCH = 512               # s-chunk (one PSUM bank of fp32)
NS = S // SCH           # 4 s-chunks per batch
NCH = BC * NS           # 32 chunks per core
U = 32                  # chunks unrolled per For_i iteration (single-iter body)

F32 = mybir.dt.float32
F32R = mybir.dt.float32r
BF16 = mybir.dt.bfloat16


def _build_program(reps=1, mode="full", unroll=U, out_eng="scalar",
                   staggered=False, dtype="f32", pipe=False):
    nc = bacc.Bacc("TRN2", target_bir_lowering=False, debug=False)

    bf = dtype == "bf16"
    mm_dt = BF16 if bf else F32R
    payload = 514 if bf else 513
    refC = nc.dram_tensor(
        "refC", [NCH, P, HT, payload], BF16 if bf else F32, kind="ExternalInput"
    ).ap()
    WrT = nc.dram_tensor(
        "WrT", [H, H], BF16 if bf else F32, kind="ExternalInput"
    ).ap()
    brt = nc.dram_tensor("brt", [P, OT], F32, kind="ExternalInput").ap()
    vt = nc.dram_tensor("vt", [P, OT], BF16, kind="ExternalInput").ap()

    refoutC = nc.dram_tensor(
        "refoutC", [NCH, P, OT, SCH], F32, kind="ExternalOutput"
    ).ap()
    logitsC = nc.dram_tensor("logitsC", [NCH, SCH], F32, kind="ExternalOutput").ap()

    Tanh = mybir.ActivationFunctionType.Tanh

    with tile.TileContext(nc) as tc:
        nbuf = min(unroll + 1, 8)
        with (
            tc.tile_pool(name="const", bufs=1) as const,
            tc.tile_pool(name="io", bufs=nbuf) as io,
            tc.tile_pool(name="outp", bufs=nbuf) as outp,
            tc.tile_pool(name="thp", bufs=3) as thp,
            tc.tile_pool(name="lgp", bufs=3) as lgp,
            tc.tile_pool(name="psum", bufs=6, space="PSUM") as psum,
            tc.tile_pool(name="psl", bufs=2, space="PSUM") as psl,
        ):
            wr_sb = const.tile([P, HT, H], mm_dt, name="wr_sb")
            nc.sync.dma_start(
                wr_sb[:], WrT.rearrange("(ht hp) o -> hp ht o", hp=P).bitcast(mm_dt)
            )
            brt_sb = const.tile([P, OT], F32, name="brt_sb")
            nc.sync.dma_start(brt_sb[:], brt)
            vt_sb = const.tile([P, OT], BF16, name="vt_sb")
            nc.sync.dma_start(vt_sb[:], vt)

            out_dma = getattr(nc, out_eng).dma_start

            if pipe:
                Tanh_ = Tanh

                def _load(pp, iv):
                    c = iv & (NCH - 1)
                    rs = pp.intermediate_tile([P, HT, payload], mm_dt)
                    nc.sync.dma_start(rs[:], refC[ds(c, 1)].bitcast(mm_dt))
                    return rs

                def _compute(pp, iv, rs):
                    out_sb = pp.intermediate_tile([P, OT, SCH], F32)
                    lg = pp.intermediate_tile([1, SCH], F32)
                    ps_l = psl.tile([1, SCH], F32, name="ps_l", tag="psl")
                    for ot in range(OT):
                        ps = psum.tile([P, SCH], F32, name="ps", tag="ps")
                        for ht in range(HT):
                            nc.tensor.matmul(
                                ps[:],
                                wr_sb[:, ht, ot * P : (ot + 1) * P],
                                rs[:, ht, 0:SCH],
                                start=(ht == 0),
                                stop=(ht == HT - 1),
                            )
                        nc.vector.tensor_scalar_add(
                            out_sb[:, ot, :], ps[:], brt_sb[:, ot : ot + 1]
                        )
                        th = thp.tile([P, SCH], BF16, name="th", tag="th")
                        nc.scalar.activation(
                            th[:], ps[:], Tanh_,
                            bias=rs[:, ot, 512:513].bitcast(
                                BF16 if bf else F32),
                        )
                        nc.tensor.matmul(
                            ps_l[:],
                            vt_sb[:, ot : ot + 1],
                            th[:],
                            start=(ot == 0),
                            stop=(ot == OT - 1),
                        )
                    nc.vector.tensor_copy(lg[:], ps_l[:])
                    return (out_sb, lg)

                def _store(pp, iv, tiles):
                    out_sb, lg = tiles
                    c = iv & (NCH - 1)
                    nc.sync.dma_start(logitsC[ds(c, 1)], lg[:])
                    out_dma(refoutC[ds(c, 1)], out_sb[:])

                tc.For_i_pipelined(
                    [_load, _compute, _store], 0, NCH * reps, 1,
                    unroll=unroll, staggered_reset=staggered,
                )
                n_iter = 0
            else:
                n_iter = (NCH // unroll) * reps
            if not pipe:
              with tc.For_i(0, n_iter, 1, staggered_reset=staggered) as iv:
                lg_big = lgp.tile([1, unroll, SCH], F32, name="lg_big",
                                  tag="lg_big", bufs=1)
                for u in range(unroll):
                    # unroll == NCH makes every chunk index static
                    c = u if unroll == NCH else (iv * unroll + u) & (NCH - 1)

                    rs = io.tile([P, HT, payload], mm_dt, name="rs", tag="rs")
                    nc.sync.dma_start(rs[:], refC[ds(c, 1)].bitcast(mm_dt))

                    if mode == "dma":
                        assert not bf
                        out_dma(
                            refoutC[ds(c, 1)], rs[:, :, 0:SCH].bitcast(F32)
                        )
                        continue

                    out_sb = outp.tile([P, OT, SCH], F32, name="out_sb",
                                       tag="out_sb")
                    ps_l = psl.tile([1, SCH], F32, name="ps_l", tag="psl")

                    for ot in range(OT):
                        ps = psum.tile([P, SCH], F32, name="ps", tag="ps")
                        for ht in range(HT):
                            nc.tensor.matmul(
                                ps[:],
                                wr_sb[:, ht, ot * P : (ot + 1) * P],
                                rs[:, ht, 0:SCH],
                                start=(ht == 0),
                                stop=(ht == HT - 1),
                            )
                        # ref_t tile = mm + br  (per-partition bias on DVE)
                        nc.vector.tensor_scalar_add(
                            out_sb[:, ot, :], ps[:], brt_sb[:, ot : ot + 1]
                        )
                        # tanh(mm + q + bq + br) on ScalarE, bias bundled in rs
                        th = thp.tile([P, SCH], BF16, name="th", tag="th")
                        nc.scalar.activation(
                            th[:],
                            ps[:],
                            Tanh,
                            bias=rs[:, ot, 512:513].bitcast(BF16 if bf else F32),
                        )
                        # logits partial: V[o-tile] . tanh  (TensorE matvec)
                        nc.tensor.matmul(
                            ps_l[:],
                            vt_sb[:, ot : ot + 1],
                            th[:],
                            start=(ot == 0),
                            stop=(ot == OT - 1),
                        )

                    nc.vector.tensor_copy(lg_big[:, u, :], ps_l[:])
                    out_dma(refoutC[ds(c, 1)], out_sb[:])

                if unroll == NCH:
                    cbase = 0
                else:
                    cbase = (iv & (NCH // unroll - 1)) * unroll
                out_dma(logitsC[ds(cbase, unroll)], lg_big[:])

    nc.compile()
    return nc


_NC_CACHE = {}


def _get_program(reps=1, mode="full", unroll=U, out_eng="scalar",
                 staggered=False, dtype="f32", pipe=False):
    key = (reps, mode, unroll, out_eng, staggered, dtype, pipe)
    if key not in _NC_CACHE:
        _NC_CACHE[key] = _build_program(reps, mode, unroll, out_eng, staggered,
                                        dtype, pipe)
    return _NC_CACHE[key]


def _prep_in_maps(query, ref, Wq, bq, Wr, br, V, dtype="f32"):
    import ml_dtypes

    bf = dtype == "bf16"
    query = np.ascontiguousarray(query, dtype=np.float32)
    ref = np.ascontiguousarray(ref, dtype=np.float32)
    Wq = np.asarray(Wq, dtype=np.float32)
    Wr = np.asarray(Wr, dtype=np.float32)
    bq = np.asarray(bq, dtype=np.float32)
    br = np.asarray(br, dtype=np.float32)
    V = np.asarray(V, dtype=np.float32)

    WrT = np.ascontiguousarray(Wr.T)
    if bf:
        WrT = WrT.astype(ml_dtypes.bfloat16)
    brt = np.ascontiguousarray(br.reshape(OT, P).T)
    vt = np.ascontiguousarray(V.reshape(OT, P).T).astype(ml_dtypes.bfloat16)

    # host-side q head (0.05% of the FLOPs)
    qbias = query @ Wq.T + bq + br                      # [B, H]

    # chunk-major ref layout: [b,s,h] -> [c=(b,si), hp, ht, s] + bias col(s)
    refT = ref.reshape(B, S, HT, P)                     # [b, s, ht, hp]
    # -> [b, si, hp, ht, s_in]
    refCm = refT.reshape(B, NS, SCH, HT, P).transpose(0, 1, 4, 3, 2)
    refCm = refCm.reshape(B * NS, P, HT, SCH)
    qb = qbias.reshape(B, OT, P).transpose(0, 2, 1)     # [b, hp, ot]
    qb = np.repeat(qb[:, None, :, :], NS, axis=1).reshape(B * NS, P, OT, 1)
    pad = [refCm, qb]
    if bf:
        pad.append(np.zeros((B * NS, P, HT, 1), np.float32))
    refC_all = np.concatenate(pad, axis=3)              # [B*NS, P, HT, 513/514]
    refC_all = np.ascontiguousarray(refC_all)
    if bf:
        refC_all = refC_all.astype(ml_dtypes.bfloat16)

    in_maps = []
    for c in range(NCORES):
        cs = slice(c * NCH, (c + 1) * NCH)
        in_maps.append(
            {"refC": refC_all[cs], "WrT": WrT, "brt": brt, "vt": vt}
        )
    return in_maps


def _gather(results):
    ref_t_parts = []
    logits_parts = []
    for c in range(NCORES):
        ro = results[c]["refoutC"]                      # [NCH, P, OT, SCH]
        # [b, si, op, ot, s] -> [b, ot, op, si, s]
        ro = ro.reshape(BC, NS, P, OT, SCH).transpose(0, 3, 2, 1, 4)
        ref_t_parts.append(ro.reshape(BC, H, S))
        logits_parts.append(results[c]["logitsC"].reshape(BC, S))
    return np.concatenate(ref_t_parts, 0), np.concatenate(logits_parts, 0)


DTYPE = "bf16"


def run_with_results(trace=False, **inputs):
    """Run on 8 NeuronCores; returns ((ref_t, logits), BassKernelResults)."""
    nc = _get_program(dtype=DTYPE)
    in_maps = _prep_in_maps(**inputs, dtype=DTYPE)
    res = run_bass_kernel_spmd(nc, in_maps, core_ids=list(range(NCORES)), trace=trace)
    ref_t, logits = _gather(res.results)
    return (ref_t, logits), res


def kernel(**inputs):
    outs, _ = run_with_results(trace=False, **inputs)
    return outs


def bench(n_iters=32, reps=1, mode="full", unroll=U, out_eng="scalar",
          staggered=False, dtype=None, pipe=False, **inputs):
    """Measure per-iteration device time by queueing n_iters back-to-back
    executions (outputs of iter i are donated as output buffers of iter
    i+1, so there is no per-iter memset/transfer)."""
    import time

    import jax
    from jax.experimental.shard_map import shard_map
    from jax.sharding import Mesh, NamedSharding, PartitionSpec

    from concourse import bass2jax
    from concourse.bass2jax import _bass_exec_p, partition_id_tensor

    if dtype is None:
        dtype = DTYPE
    nc = _get_program(reps, mode, unroll, out_eng, staggered, dtype, pipe)
    in_maps = _prep_in_maps(**inputs, dtype=dtype)
    bass2jax.install_neuronx_cc_hook()

    import concourse.mybir as _mybir

    partition_name = nc.partition_id_tensor.name if nc.partition_id_tensor else None
    in_names, out_names, out_avals, zero_outs = [], [], [], []
    for alloc in nc.m.functions[0].allocations:
        if not isinstance(alloc, _mybir.MemoryLocationSet):
            continue
        name = alloc.memorylocations[0].name
        if alloc.kind == "ExternalInput":
            if name != partition_name:
                in_names.append(name)
        elif alloc.kind == "ExternalOutput":
            shape = tuple(alloc.tensor_shape)
            dtype = _mybir.dt.np(alloc.dtype)
            out_names.append(name)
            out_avals.append(jax.core.ShapedArray(shape, dtype))
            zero_outs.append(np.zeros(shape, dtype))
    n_params = len(in_names)
    n_outs = len(out_avals)
    in_names = in_names + out_names
    if partition_name is not None:
        in_names.append(partition_name)

    def _body(*args):
        operands = list(args)
        if partition_name is not None:
            operands.append(partition_id_tensor())
        return tuple(
            _bass_exec_p.bind(
                *operands,
                out_avals=tuple(out_avals),
                in_names=tuple(in_names),
                out_names=tuple(out_names),
                lowering_input_output_aliases=(),
                sim_require_finite=True,
                sim_require_nnan=True,
                nc=nc,
            )
        )

    devices = jax.devices()[:NCORES]
    mesh = Mesh(np.asarray(devices), ("core",))
    in_specs = (PartitionSpec("core"),) * (n_params + n_outs)
    out_specs = (PartitionSpec("core"),) * n_outs
    donate = tuple(range(n_params, n_params + n_outs))
    sharded = jax.jit(
        shard_map(_body, mesh=mesh, in_specs=in_specs, out_specs=out_specs,
                  check_rep=False),
        donate_argnums=donate,
        keep_unused=True,
    )

    per_core = [
        [np.asarray(m[name]) for name in in_names[:n_params]] for m in in_maps
    ]
    concat_in = [
        np.concatenate([per_core[c][i] for c in range(NCORES)], axis=0)
        for i in range(n_params)
    ]
    concat_zeros = [
        np.zeros((NCORES * z.shape[0], *z.shape[1:]), z.dtype) for z in zero_outs
    ]
    shard = NamedSharding(mesh, PartitionSpec("core"))
    concat_in = [jax.device_put(a, shard) for a in concat_in]
    outs = sharded(*concat_in, *concat_zeros)  # warmup + compile
    jax.block_until_ready(outs)

    t0 = time.time()
    for _ in range(n_iters):
        outs = sharded(*concat_in, *outs)
    jax.block_until_ready(outs)
    t1 = time.time()
    return (t1 - t0) / n_iters
